# revision 13
# baseline (speedup 1.0000x reference)
"""DeepHisCoM Trainium2 kernel (nn_DeepHisCoM_7017976562218).

Math (reference):
    xr = x.reshape(B, P, V)
    z1 = einsum('bpv,pwv->bpw', xr, W1);  h = leaky(z1)          # per-pathway Linear V->W
    z2 = einsum('bpw,pw->bp', h, W2);     pval = leaky(z2)       # per-pathway Linear W->1
    BN(batch stats) -> global L2 normalize -> sigmoid(pn @ Wd + bd)

Decomposition: with wt = 0.8*W2 and q = w2^T z1 (linear in x),
    z2 = 0.2*q + sum_t wt_t * relu(z1_t)
       = 0.6*q + 0.5 * sum_t sign(wt_t) * |wt_t * z1_t|          # relu(x) = (x+|x|)/2
The |.| form needs no relu engine pass and no sign bookkeeping on device:
columns are pre-scaled by |wt| on the host and laid out as a positive rail
and a negative rail; the device reduces  t1 = sum(pos rail) - sum(neg rail)
of |h~| and the host adds the 0.6*q linear term (one tiny batched GEMV,
0.17% of the model FLOPs, computed alongside BN/L2/sigmoid).

Device strategy (8 NeuronCores, batch-sharded 2048 rows/core), three
pathway routes chosen per-pathway by sign balance (host-planned from W2):
    - scan route (the ~48 most sign-unbalanced pathways): 64-col matmul
      (W1^T * S), VectorE fused prefix-sum of w2*relu(h) (custom DVE op),
      per-pathway sums via segment-boundary differences on GpSimd.
    - fold route (the ~66 most balanced): matmul of |wt|-prescaled rails
      [pos R | neg R]; ScalarE |.|-drains PSUM to bf16; VectorE runs a
      rail-difference prefix scan (cumsum(pos - neg), custom DVE op) over
      R cols/pathway instead of 64 -> ~2x less VectorE work; boundary
      differences on GpSimd give t1 directly.
    - offload route (~14 mid pathways): rails as above, but GpSimd
      tree-folds the |h~| rails 4:1 and ships fp8 partials to the host.
All routes write raw (pre-leaky) z2 parts; host applies leaky + BN.
"""

import os
import sys

import numpy as np

for _p in ("/opt/trn_rl_repo",):
    if _p not in sys.path and os.path.isdir(_p):
        sys.path.insert(0, _p)

import ml_dtypes

import concourse.bacc as bacc
import concourse.bass as bass
import concourse.mybir as mybir
from concourse import dve_ops
from concourse.bass_utils import run_bass_kernel_spmd
from concourse.dve_spec import AluOp, Spec, Src0, Src1, Zero, relu, scan
from concourse.tile import TileContext


def _register_op(name, spec_body, ref, sha):
    for op in dve_ops.OPS:
        if op.name == name:
            return op
    op = dve_ops.DveOp(name, Spec(body=spec_body, reference=ref),
                       subdim=False, uops_sha=sha)
    dve_ops.OPS.append(op)
    dve_ops._SUB_OPCODE_FOR_NAME[name] = dve_ops._CUSTOM_DVE_ROW_BASE + len(dve_ops.OPS) - 1
    dve_ops.CUSTOM_DVE_SPECS[name] = op.spec
    return op


# out[t] = running sum of in0[t] - in1[t]   (pos rail minus neg rail)
RAIL_DIFF_OP = _register_op(
    "RAIL_DIFF_SCAN_ANT",
    scan(AluOp.ADD, Src0 - Src1, init=Zero),
    lambda in0, in1, s0, s1, imm2: np.cumsum(
        in0.astype(np.float32) - in1.astype(np.float32), axis=-1),
    {"v3": "1a1aa71c909c123d", "v4": "cc23e684b94fd50d"},
)
# out[t] = running sum of in0[t] * relu(in1[t])
PREFIX_SUM_OP = _register_op(
    "STT_PREFIX_SUM_ANT",
    scan(AluOp.ADD, Src0 * relu(Src1), init=Zero),
    lambda in0, in1, s0, s1, imm2: np.cumsum(
        in0.astype(np.float32) * np.maximum(in1, 0), axis=-1),
    {"v3": "0179e875ac56dbc9", "v4": "d52b99774727e4db"},
)

P, V, W = 128, 128, 64
B = 16384
N_CORES = 8
BSH = B // N_CORES          # 2048 batch rows per core
NBT = BSH // 128            # 16 batch tiles per core
BN_EPS = 1e-5
F32 = mybir.dt.float32
BF16 = mybir.dt.bfloat16
F8 = mybir.dt.float8e4
S_SCAN = 8.0                # fp8 scale for scan-route W1^T columns
S_RAIL = 128.0              # fp8 scale for |wt|-prescaled rail columns

# per-half route sizes (128 pathways total, 64 per half)
NF = 28                     # fold supertile pathways (4 PSUM banks, 7/bank)
NS = 24                     # scan pathways (3 banks x 8, T=64, zero waste)
NO = 7                      # offload pathways (1 bank, rails)
NC = 5                      # small fold tile (1 bank)
R_OFF = 36                  # offload rail width (even, for the 2:1 tree)
OFF_SHIP = 2 * (R_OFF // 4) # fp8 partials shipped per offl pathway (18)

_CACHE = {}
LAST_RESULTS = None


def _plan(W2):
    """Route assignment + rail widths from the sign balance of W2."""
    wt = 0.8 * np.asarray(W2, np.float32)
    npos = (wt > 0).sum(1)
    bal = np.maximum(npos, W - npos)
    order = np.argsort(bal, kind="stable")
    fold = order[: 2 * (NF + NC)]
    offl = order[2 * (NF + NC) : 2 * (NF + NC + NO)]
    scn = order[2 * (NF + NC + NO) :]
    halves = []
    for h in range(2):
        f_main = fold[h * NF : (h + 1) * NF]
        f_small = fold[2 * NF + h * NC : 2 * NF + (h + 1) * NC]
        s_h = scn[h * NS : (h + 1) * NS]
        o_h = offl[h * NO : (h + 1) * NO]
        r_main = int(bal[f_main].max())
        r_small = int(bal[f_small].max())
        assert 7 * 2 * r_main <= 512 and 2 * r_small <= 512 and 2 * R_OFF >= 2 * int(bal[o_h].max())
        halves.append({
            "f_main": f_main, "r_main": r_main,
            "f_small": f_small, "r_small": r_small,
            "scan": s_h, "offl": o_h,
        })
    return halves


def _geom_sig(halves):
    return tuple((h["r_main"], h["r_small"]) for h in halves)


def _build_program(halves):
    nc = bacc.Bacc()
    wext_cols = sum(
        h["r_main"] * 2 * NF + h["r_small"] * 2 * NC + NS * W + NO * 2 * R_OFF
        for h in halves
    )
    xt_in = nc.declare_dram_parameter("xt", [V, BSH * P], F8, isOutput=False)
    wext_in = nc.declare_dram_parameter("wext", [V, wext_cols], F8, isOutput=False)
    w2e_in = nc.declare_dram_parameter("w2ext", [128, 2 * NS * W], BF16, isOutput=False)
    p_out = nc.declare_dram_parameter("ps", [BSH, P], BF16, isOutput=True)
    h_out = nc.declare_dram_parameter("hs", [BSH, 2 * NO * OFF_SHIP], F8, isOutput=True)

    # per-half wext column offsets: [F(28 rails) | scan(24*64) | offl(7*72) | C(5 rails)]
    wo = []
    off = 0
    for h in range(2):
        g = halves[h]
        d = {"F": off}
        off += 2 * g["r_main"] * NF
        d["S"] = off
        off += NS * W
        d["O"] = off
        off += NO * 2 * R_OFF
        d["C"] = off
        off += 2 * g["r_small"] * NC
        wo.append(d)
    assert off == wext_cols

    with TileContext(nc) as tc:
        with (
            tc.tile_pool(name="singles", bufs=1) as singles,
            tc.tile_pool(name="xt", bufs=3) as xtp,
            tc.tile_pool(name="prod", bufs=3) as prodp,
            tc.tile_pool(name="habs", bufs=3) as habsp,
            tc.tile_pool(name="hsb", bufs=2) as hsbp,
            tc.tile_pool(name="pf", bufs=2) as pfp,
            tc.tile_pool(name="hps", bufs=2, space="PSUM") as hpsp,
        ):
            wext = singles.tile([V, wext_cols], F8)
            w2e = singles.tile([128, 2 * NS * W], BF16)
            # weights on the scalar HWDGE ring; first fold + scan cols of both
            # halves land first so the pipeline ramps immediately
            for h in range(2):
                c0 = wo[h]["F"]
                nc.scalar.dma_start(out=wext[:, c0 : c0 + 2 * halves[h]["r_main"] * NF],
                                    in_=wext_in[:, c0 : c0 + 2 * halves[h]["r_main"] * NF])
                nc.scalar.dma_start(out=w2e[:, h * NS * W : (h + 1) * NS * W],
                                    in_=w2e_in[:, h * NS * W : (h + 1) * NS * W])
            for h in range(2):
                c0, c1 = wo[h]["S"], wo[h]["C"] + 2 * halves[h]["r_small"] * NC
                nc.scalar.dma_start(out=wext[:, c0:c1], in_=wext_in[:, c0:c1])

            for bt in range(NBT):
                xt = xtp.tile([128, 128 * 128], F8, tag="xt")
                base_col = bt * 128 * 128
                bounds = (0, 1792, 4096, 8192, 16384) if bt == 0 \
                    else (0, 8192, 16384)
                for c0, c1 in zip(bounds[:-1], bounds[1:]):
                    nc.sync.dma_start(
                        out=xt[:, c0:c1],
                        in_=xt_in[:, base_col + c0 : base_col + c1],
                    )
                if bt % 2 == 0:
                    pf = pfp.tile([128, 2 * P], BF16, tag="pf")
                    hsb = hsbp.tile([128, 2 * 2 * NO * OFF_SHIP], F8, tag="hsb")
                po = (bt % 2) * P
                hso = (bt % 2) * 2 * NO * OFF_SHIP
                for half in range(2):
                    g = halves[half]
                    xoff = half * 64 * 128
                    slot0 = half * 64
                    Rm, Rs = g["r_main"], g["r_small"]

                    def mm(ps_ap, slot, cols0, ncol):
                        nc.tensor.matmul(
                            ps_ap,
                            lhsT=xt[:, xoff + slot * 128 : xoff + (slot + 1) * 128],
                            rhs=wext[:, cols0 : cols0 + ncol],
                            start=True, stop=True,
                        )

                    # ---- fold supertile: 28 pathways, rails [pos Rm | neg Rm]
                    Tm = 2 * Rm
                    h_ps = hpsp.tile([128, 2048], F32)
                    for j in range(NF):
                        o = (j // 7) * 512 + (j % 7) * Tm
                        mm(h_ps[:, o : o + Tm], j, wo[half]["F"] + j * Tm, Tm)
                    habs = habsp.tile([128, NF * Tm], BF16)
                    h4 = h_ps[:].rearrange("p (b q) -> p b q", b=4)[:, :, : 7 * Tm]
                    a4 = habs[:].rearrange("p (b q) -> p b q", b=4)
                    nc.scalar.activation(out=a4, in_=h4,
                                         func=mybir.ActivationFunctionType.Abs)
                    prod = prodp.tile([128, (NF + 1) * Rm], F32)
                    nc.gpsimd.memset(prod[:, Rm - 1 : Rm], 0.0)
                    hr = habs[:].rearrange("p (j r) -> p j r", r=Tm)
                    nc.vector._custom_dve(
                        RAIL_DIFF_OP,
                        out=prod[:, Rm:].rearrange("p (j r) -> p j r", r=Rm),
                        in0=hr[:, :, :Rm], in1=hr[:, :, Rm:],
                    )
                    ends = prod[:].rearrange("p (j r) -> p j r", r=Rm)[
                        :, :, Rm - 1 : Rm].rearrange("p j r -> p (j r)")
                    nc.gpsimd.tensor_sub(
                        out=pf[:, po + slot0 : po + slot0 + NF],
                        in0=ends[:, 1 : NF + 1], in1=ends[:, 0:NF],
                    )

                    # ---- mixed tile: scan 24 (banks 0-2, T=64) + offl 7 (bank 3)
                    h_ps = hpsp.tile([128, 2048], F32)
                    for j in range(NS):
                        o = (j // 8) * 512 + (j % 8) * W
                        mm(h_ps[:, o : o + W], NF + j, wo[half]["S"] + j * W, W)
                    for j in range(NO):
                        o = 3 * 512 + j * 2 * R_OFF
                        mm(h_ps[:, o : o + 2 * R_OFF],
                           NF + NS + j, wo[half]["O"] + j * 2 * R_OFF, 2 * R_OFF)
                    # scan: prefix-sum of w2*relu(h) over banks 0-2
                    prod = prodp.tile([128, (NS + 1) * W], F32)
                    nc.gpsimd.memset(prod[:, W - 1 : W], 0.0)
                    nc.vector._custom_dve(
                        PREFIX_SUM_OP,
                        out=prod[:, W:].rearrange("p (b c) -> p b c", b=3),
                        in0=w2e[:, half * NS * W : (half + 1) * NS * W].rearrange(
                            "p (b c) -> p b c", b=3),
                        in1=h_ps[:].rearrange("p (b q) -> p b q", b=4)[:, :3, : 8 * W],
                    )
                    ends = prod[:].rearrange("p (j c) -> p j c", c=W)[
                        :, :, W - 1 : W].rearrange("p j c -> p (j c)")
                    nc.gpsimd.tensor_sub(
                        out=pf[:, po + slot0 + NF : po + slot0 + NF + NS],
                        in0=ends[:, 1 : NS + 1], in1=ends[:, 0:NS],
                    )
                    # offl: |.|-drain bank 3, 4:1 tree on GpSimd, fp8 partials
                    habs = habsp.tile([128, NO * 2 * R_OFF + NO * R_OFF], BF16)
                    nc.scalar.activation(
                        out=habs[:, : NO * 2 * R_OFF],
                        in_=h_ps[:, 3 * 512 : 3 * 512 + NO * 2 * R_OFF],
                        func=mybir.ActivationFunctionType.Abs,
                    )
                    t0 = habs[:, : NO * 2 * R_OFF].rearrange(
                        "p (j k r) -> p j k r", j=NO, k=2)
                    t1 = habs[:, NO * 2 * R_OFF :].rearrange(
                        "p (j k r) -> p j k r", j=NO, k=2)
                    nc.gpsimd.tensor_add(out=t1, in0=t0[:, :, :, : R_OFF // 2],
                                         in1=t0[:, :, :, R_OFF // 2 :])
                    sc0 = hso + half * NO * OFF_SHIP
                    ship = hsb[:, sc0 : sc0 + NO * OFF_SHIP]
                    s4 = ship.rearrange("p (j k r) -> p j k r", j=NO, k=2)
                    nc.gpsimd.tensor_add(out=s4, in0=t1[:, :, :, : R_OFF // 4],
                                         in1=t1[:, :, :, R_OFF // 4 :])

                    # ---- small fold tile: 5 pathways, 1 bank
                    Ts = 2 * Rs
                    h_ps = hpsp.tile([128, NC * Ts], F32)
                    for j in range(NC):
                        mm(h_ps[:, j * Ts : (j + 1) * Ts],
                           NF + NS + NO + j, wo[half]["C"] + j * Ts, Ts)
                    habs = habsp.tile([128, NC * Ts], BF16)
                    nc.scalar.activation(out=habs[:], in_=h_ps[:],
                                         func=mybir.ActivationFunctionType.Abs)
                    prod = prodp.tile([128, (NC + 1) * Rs], F32)
                    nc.gpsimd.memset(prod[:, Rs - 1 : Rs], 0.0)
                    hr = habs[:].rearrange("p (j r) -> p j r", r=Ts)
                    nc.vector._custom_dve(
                        RAIL_DIFF_OP,
                        out=prod[:, Rs:].rearrange("p (j r) -> p j r", r=Rs),
                        in0=hr[:, :, :Rs], in1=hr[:, :, Rs:],
                    )
                    ends = prod[:].rearrange("p (j r) -> p j r", r=Rs)[
                        :, :, Rs - 1 : Rs].rearrange("p j r -> p (j r)")
                    nc.gpsimd.tensor_sub(
                        out=pf[:, po + slot0 + NF + NS + NO :][:, :NC],
                        in0=ends[:, 1 : NC + 1], in1=ends[:, 0:NC],
                    )
                # batched stores every 2 bt on the scalar ring
                if bt % 2 == 1:
                    nc.scalar.dma_start(
                        out=p_out[(bt - 1) * 128 : (bt + 1) * 128, :].rearrange(
                            "(k p) c -> p k c", p=128),
                        in_=pf[:].rearrange("p (k c) -> p k c", k=2),
                    )
                    nc.sync.dma_start(
                        out=h_out[(bt - 1) * 128 : (bt + 1) * 128, :].rearrange(
                            "(k p) c -> p k c", p=128),
                        in_=hsb[:].rearrange("p (k c) -> p k c", k=2),
                    )
    nc.finalize()
    return nc, wext_cols, wo


def _prep(W1, W2, halves, wext_cols, wo):
    W1T = np.transpose(np.asarray(W1, np.float32), (0, 2, 1))        # [P,V,W]
    wt = 0.8 * np.asarray(W2, np.float32)                            # [P,W]
    wext = np.zeros((V, wext_cols), np.float32)
    w2e_row = np.zeros(2 * NS * W, np.float32)

    def rails(p, R):
        pos = np.where(wt[p] > 0)[0]
        neg = np.where(wt[p] <= 0)[0]
        blk = np.zeros((V, 2 * R), np.float32)
        blk[:, : len(pos)] = W1T[p][:, pos] * np.abs(wt[p][pos])
        blk[:, R : R + len(neg)] = W1T[p][:, neg] * np.abs(wt[p][neg])
        return blk * S_RAIL

    for h in range(2):
        g = halves[h]
        c = wo[h]["F"]
        for p in g["f_main"]:
            wext[:, c : c + 2 * g["r_main"]] = rails(p, g["r_main"])
            c += 2 * g["r_main"]
        c = wo[h]["S"]
        for i, p in enumerate(g["scan"]):
            wext[:, c : c + W] = W1T[p] * S_SCAN
            w2e_row[h * NS * W + i * W : h * NS * W + (i + 1) * W] = wt[p]
            c += W
        c = wo[h]["O"]
        for p in g["offl"]:
            wext[:, c : c + 2 * R_OFF] = rails(p, R_OFF)
            c += 2 * R_OFF
        c = wo[h]["C"]
        for p in g["f_small"]:
            wext[:, c : c + 2 * g["r_small"]] = rails(p, g["r_small"])
            c += 2 * g["r_small"]
    wext_f8 = wext.astype(ml_dtypes.float8_e4m3)
    w2ext = np.ascontiguousarray(np.broadcast_to(
        w2e_row.astype(ml_dtypes.bfloat16)[None, :], (128, 2 * NS * W)))
    return wext_f8, w2ext


def _prep_xt(x_f8, slot_paths):
    """Pre-transpose per core into [v, (bt, slot, b)] fp8, slot = device order."""
    out = []
    for c in range(N_CORES):
        xc = x_f8[c * BSH : (c + 1) * BSH, :]
        xt = (
            xc.reshape(NBT, 128, P, V)[:, :, slot_paths, :]  # [bt, b, slot, v]
            .transpose(3, 0, 2, 1)                           # [v, bt, slot, b]
            .reshape(V, BSH * P)
        )
        out.append(np.ascontiguousarray(xt))
    return out


def kernel(x, W1, W2, gamma, beta, Wd, bd):
    global LAST_RESULTS
    x = np.ascontiguousarray(np.asarray(x, dtype=np.float32))
    W1 = np.asarray(W1, dtype=np.float32)
    W2 = np.asarray(W2, dtype=np.float32)

    halves = _plan(W2)
    sig = _geom_sig(halves)
    if _CACHE.get("sig") != sig:
        _CACHE["nc"], _CACHE["wc"], _CACHE["wo"] = _build_program(halves)
        _CACHE["sig"] = sig
    nc, wext_cols, wo = _CACHE["nc"], _CACHE["wc"], _CACHE["wo"]

    # device slot order -> original pathway ids
    slot_paths = np.concatenate([
        np.concatenate([g["f_main"], g["scan"], g["offl"], g["f_small"]])
        for g in halves
    ]).astype(np.int64)

    wext_f8, w2ext = _prep(W1, W2, halves, wext_cols, wo)
    x_f8 = x.astype(ml_dtypes.float8_e4m3)
    xts = _prep_xt(x_f8, slot_paths)
    in_maps = [
        {"xt": xts[c], "wext": wext_f8, "w2ext": w2ext}
        for c in range(N_CORES)
    ]
    res = run_bass_kernel_spmd(nc, in_maps, list(range(N_CORES)))
    LAST_RESULTS = res

    # ---- host finish ----
    # linear term q[b,p] = w2_p . z1 = x_bp . (W1T_p @ w2_p), batched BLAS
    W1T = np.transpose(W1, (0, 2, 1))
    U2 = np.einsum("pvw,pw->pv", W1T, W2).astype(np.float32)          # [P,V]
    xr = x.reshape(B, P, V)
    qall = np.einsum("bpv,pv->bp", xr, U2, optimize=True).astype(np.float64)

    ps = np.concatenate([res.results[c]["ps"] for c in range(N_CORES)],
                        axis=0).astype(np.float64)                    # [B,P] slots
    hs = np.concatenate([res.results[c]["hs"] for c in range(N_CORES)],
                        axis=0).astype(np.float64)                    # [B, 2*NO*18]

    z2 = np.empty((B, P), np.float64)
    for h in range(2):
        g = halves[h]
        s0 = h * 64
        # fold: z2 = 0.6 q + 0.5 * t1 / S_RAIL
        for idx, plist in ((0, g["f_main"]), (NF + NS + NO, g["f_small"])):
            sl = ps[:, s0 + idx : s0 + idx + len(plist)]
            z2[:, plist] = 0.6 * qall[:, plist] + 0.5 * sl / S_RAIL
        # scan: z2 = 0.2 q + slot / S_SCAN
        sl = ps[:, s0 + NF : s0 + NF + NS]
        z2[:, g["scan"]] = 0.2 * qall[:, g["scan"]] + sl / S_SCAN
        # offl: shipped fp8 partials [NO, 2, 18/2] pos/neg
        blk = hs[:, h * NO * OFF_SHIP : (h + 1) * NO * OFF_SHIP].reshape(
            B, NO, 2, OFF_SHIP // 2)
        t1 = blk[:, :, 0, :].sum(-1) - blk[:, :, 1, :].sum(-1)
        z2[:, g["offl"]] = 0.6 * qall[:, g["offl"]] + 0.5 * t1 / S_RAIL

    pvals = np.where(z2 >= 0, z2, 0.2 * z2)
    mean = pvals.mean(axis=0)
    var = pvals.var(axis=0)
    pn = (pvals - mean) / np.sqrt(var + BN_EPS) * np.asarray(gamma, np.float64) \
        + np.asarray(beta, np.float64)
    pn = pn / np.linalg.norm(pn)
    out = 1.0 / (1.0 + np.exp(-(pn @ np.asarray(Wd, np.float64)
                                + np.asarray(bd, np.float64))))
    return out.astype(np.float32)


# revision 14
# speedup vs baseline: 1.0212x; 1.0212x over previous
"""DeepHisCoM Trainium2 kernel (nn_DeepHisCoM_7017976562218).

Math (reference):
    xr = x.reshape(B, P, V)
    z1 = einsum('bpv,pwv->bpw', xr, W1);  h = leaky(z1)          # per-pathway Linear V->W
    z2 = einsum('bpw,pw->bp', h, W2);     pval = leaky(z2)       # per-pathway Linear W->1
    BN(batch stats) -> global L2 normalize -> sigmoid(pn @ Wd + bd)

Decomposition: with wt = 0.8*W2 and q = w2^T z1 (linear in x),
    z2 = 0.2*q + sum_t wt_t * relu(z1_t)
       = 0.6*q + 0.5 * sum_t sign(wt_t) * |wt_t * z1_t|          # relu(x) = (x+|x|)/2
The |.| form needs no relu engine pass and no sign bookkeeping on device:
columns are pre-scaled by |wt| on the host and laid out as a positive rail
and a negative rail; the device reduces  t1 = sum(pos rail) - sum(neg rail)
of |h~| and the host adds the 0.6*q linear term (one tiny batched GEMV,
0.17% of the model FLOPs, computed alongside BN/L2/sigmoid).

Device strategy (8 NeuronCores, batch-sharded 2048 rows/core), three
pathway routes chosen per-pathway by sign balance (host-planned from W2):
    - scan route (the ~48 most sign-unbalanced pathways): 64-col matmul
      (W1^T * S), VectorE fused prefix-sum of w2*relu(h) (custom DVE op),
      per-pathway sums via segment-boundary differences on GpSimd.
    - fold route (the ~66 most balanced): matmul of |wt|-prescaled rails
      [pos R | neg R]; ScalarE |.|-drains PSUM to bf16; VectorE runs a
      rail-difference prefix scan (cumsum(pos - neg), custom DVE op) over
      R cols/pathway instead of 64 -> ~2x less VectorE work; boundary
      differences on GpSimd give t1 directly.
    - offload route (~14 mid pathways): rails as above, but GpSimd
      tree-folds the |h~| rails 4:1 and ships fp8 partials to the host.
All routes write raw (pre-leaky) z2 parts; host applies leaky + BN.
"""

import os
import sys

import numpy as np

for _p in ("/opt/trn_rl_repo",):
    if _p not in sys.path and os.path.isdir(_p):
        sys.path.insert(0, _p)

import ml_dtypes

import concourse.bacc as bacc
import concourse.bass as bass
import concourse.mybir as mybir
from concourse import dve_ops
from concourse.bass_utils import run_bass_kernel_spmd
from concourse.dve_spec import AluOp, Spec, Src0, Src1, Zero, relu, scan
from concourse.tile import TileContext


def _register_op(name, spec_body, ref, sha):
    for op in dve_ops.OPS:
        if op.name == name:
            return op
    op = dve_ops.DveOp(name, Spec(body=spec_body, reference=ref),
                       subdim=False, uops_sha=sha)
    dve_ops.OPS.append(op)
    dve_ops._SUB_OPCODE_FOR_NAME[name] = dve_ops._CUSTOM_DVE_ROW_BASE + len(dve_ops.OPS) - 1
    dve_ops.CUSTOM_DVE_SPECS[name] = op.spec
    return op


# out[t] = running sum of in0[t] - in1[t]   (pos rail minus neg rail)
RAIL_DIFF_OP = _register_op(
    "RAIL_DIFF_SCAN_ANT",
    scan(AluOp.ADD, Src0 - Src1, init=Zero),
    lambda in0, in1, s0, s1, imm2: np.cumsum(
        in0.astype(np.float32) - in1.astype(np.float32), axis=-1),
    {"v3": "1a1aa71c909c123d", "v4": "cc23e684b94fd50d"},
)
# out[t] = running sum of in0[t] * relu(in1[t])
PREFIX_SUM_OP = _register_op(
    "STT_PREFIX_SUM_ANT",
    scan(AluOp.ADD, Src0 * relu(Src1), init=Zero),
    lambda in0, in1, s0, s1, imm2: np.cumsum(
        in0.astype(np.float32) * np.maximum(in1, 0), axis=-1),
    {"v3": "0179e875ac56dbc9", "v4": "d52b99774727e4db"},
)

P, V, W = 128, 128, 64
B = 16384
N_CORES = 8
BSH = B // N_CORES          # 2048 batch rows per core
NBT = BSH // 128            # 16 batch tiles per core
BN_EPS = 1e-5
F32 = mybir.dt.float32
BF16 = mybir.dt.bfloat16
F8 = mybir.dt.float8e4
S_SCAN = 8.0                # fp8 scale for scan-route W1^T columns
S_RAIL = 128.0              # fp8 scale for |wt|-prescaled rail columns

# per-half route sizes (128 pathways total, 64 per half)
NF = 28                     # fold supertile pathways (4 PSUM banks, 7/bank)
NS = 24                     # scan pathways (3 banks x 8, T=64, zero waste)
NO = 7                      # offload pathways (1 bank, rails)
NC = 5                      # small fold tile (1 bank)
R_OFF = 36                  # offload rail width (even, for the 2:1 tree)
OFF_SHIP = 2 * (R_OFF // 4) # fp8 partials shipped per offl pathway (18)

_CACHE = {}
LAST_RESULTS = None


def _plan(W2):
    """Route assignment + rail widths from the sign balance of W2."""
    wt = 0.8 * np.asarray(W2, np.float32)
    npos = (wt > 0).sum(1)
    bal = np.maximum(npos, W - npos)
    order = np.argsort(bal, kind="stable")
    fold = order[: 2 * (NF + NC)]
    offl = order[2 * (NF + NC) : 2 * (NF + NC + NO)]
    scn = order[2 * (NF + NC + NO) :]
    halves = []
    for h in range(2):
        f_main = fold[h * NF : (h + 1) * NF]
        f_small = fold[2 * NF + h * NC : 2 * NF + (h + 1) * NC]
        s_h = scn[h * NS : (h + 1) * NS]
        o_h = offl[h * NO : (h + 1) * NO]
        r_main = int(bal[f_main].max())
        r_small = int(bal[f_small].max())
        assert 7 * 2 * r_main <= 512 and 2 * r_small <= 512 and 2 * R_OFF >= 2 * int(bal[o_h].max())
        halves.append({
            "f_main": f_main, "r_main": r_main,
            "f_small": f_small, "r_small": r_small,
            "scan": s_h, "offl": o_h,
        })
    return halves


def _geom_sig(halves):
    return tuple((h["r_main"], h["r_small"]) for h in halves)


def _build_program(halves):
    nc = bacc.Bacc()
    wext_cols = sum(
        h["r_main"] * 2 * NF + h["r_small"] * 2 * NC + NS * W + NO * 2 * R_OFF
        for h in halves
    )
    xt_in = nc.declare_dram_parameter("xt", [V, BSH * P], F8, isOutput=False)
    wext_in = nc.declare_dram_parameter("wext", [V, wext_cols], F8, isOutput=False)
    w2e_in = nc.declare_dram_parameter("w2ext", [128, 2 * NS * W], BF16, isOutput=False)
    p_out = nc.declare_dram_parameter("ps", [BSH, P], BF16, isOutput=True)
    h_out = nc.declare_dram_parameter("hs", [BSH, 2 * NO * OFF_SHIP], F8, isOutput=True)

    # per-half wext column offsets: [F(28 rails) | scan(24*64) | offl(7*72) | C(5 rails)]
    wo = []
    off = 0
    for h in range(2):
        g = halves[h]
        d = {"F": off}
        off += 2 * g["r_main"] * NF
        d["S"] = off
        off += NS * W
        d["O"] = off
        off += NO * 2 * R_OFF
        d["C"] = off
        off += 2 * g["r_small"] * NC
        wo.append(d)
    assert off == wext_cols

    with TileContext(nc) as tc:
        with (
            tc.tile_pool(name="singles", bufs=1) as singles,
            tc.tile_pool(name="xt", bufs=4) as xtp,
            tc.tile_pool(name="prod", bufs=6) as prodp,
            tc.tile_pool(name="habs", bufs=6) as habsp,
            tc.tile_pool(name="hsb", bufs=2) as hsbp,
            tc.tile_pool(name="pf", bufs=2) as pfp,
            tc.tile_pool(name="hps", bufs=2, space="PSUM") as hpsp,
        ):
            wext = singles.tile([V, wext_cols], F8)
            w2e = singles.tile([128, 2 * NS * W], BF16)
            # weights on the scalar HWDGE ring; first fold + scan cols of both
            # halves land first so the pipeline ramps immediately
            for h in range(2):
                c0 = wo[h]["F"]
                nc.scalar.dma_start(out=wext[:, c0 : c0 + 2 * halves[h]["r_main"] * NF],
                                    in_=wext_in[:, c0 : c0 + 2 * halves[h]["r_main"] * NF])
                nc.scalar.dma_start(out=w2e[:, h * NS * W : (h + 1) * NS * W],
                                    in_=w2e_in[:, h * NS * W : (h + 1) * NS * W])
            for h in range(2):
                c0, c1 = wo[h]["S"], wo[h]["C"] + 2 * halves[h]["r_small"] * NC
                nc.scalar.dma_start(out=wext[:, c0:c1], in_=wext_in[:, c0:c1])

            for bt in range(NBT):
                xt = xtp.tile([128, 128 * 128], F8, tag="xt")
                base_col = bt * 128 * 128
                bounds = (0, 1792, 4096, 8192, 16384) if bt == 0 \
                    else (0, 8192, 16384)
                for c0, c1 in zip(bounds[:-1], bounds[1:]):
                    nc.sync.dma_start(
                        out=xt[:, c0:c1],
                        in_=xt_in[:, base_col + c0 : base_col + c1],
                    )
                if bt % 2 == 0:
                    pf = pfp.tile([128, 2 * P], BF16, tag="pf")
                    hsb = hsbp.tile([128, 2 * 2 * NO * OFF_SHIP], F8, tag="hsb")
                po = (bt % 2) * P
                hso = (bt % 2) * 2 * NO * OFF_SHIP
                for half in range(2):
                    g = halves[half]
                    xoff = half * 64 * 128
                    slot0 = half * 64
                    Rm, Rs = g["r_main"], g["r_small"]

                    def mm(ps_ap, slot, cols0, ncol):
                        nc.tensor.matmul(
                            ps_ap,
                            lhsT=xt[:, xoff + slot * 128 : xoff + (slot + 1) * 128],
                            rhs=wext[:, cols0 : cols0 + ncol],
                            start=True, stop=True,
                        )

                    # ---- fold supertile: 28 pathways, rails [pos Rm | neg Rm]
                    Tm = 2 * Rm
                    h_ps = hpsp.tile([128, 2048], F32)
                    for j in range(NF):
                        o = (j // 7) * 512 + (j % 7) * Tm
                        mm(h_ps[:, o : o + Tm], j, wo[half]["F"] + j * Tm, Tm)
                    habs = habsp.tile([128, NF * Tm], BF16)
                    h4 = h_ps[:].rearrange("p (b q) -> p b q", b=4)[:, :, : 7 * Tm]
                    a4 = habs[:].rearrange("p (b q) -> p b q", b=4)
                    nc.scalar.activation(out=a4, in_=h4,
                                         func=mybir.ActivationFunctionType.Abs)
                    prod = prodp.tile([128, (NF + 1) * Rm], F32)
                    nc.gpsimd.memset(prod[:, Rm - 1 : Rm], 0.0)
                    hr = habs[:].rearrange("p (j r) -> p j r", r=Tm)
                    nc.vector._custom_dve(
                        RAIL_DIFF_OP,
                        out=prod[:, Rm:].rearrange("p (j r) -> p j r", r=Rm),
                        in0=hr[:, :, :Rm], in1=hr[:, :, Rm:],
                    )
                    ends = prod[:].rearrange("p (j r) -> p j r", r=Rm)[
                        :, :, Rm - 1 : Rm].rearrange("p j r -> p (j r)")
                    nc.gpsimd.tensor_sub(
                        out=pf[:, po + slot0 : po + slot0 + NF],
                        in0=ends[:, 1 : NF + 1], in1=ends[:, 0:NF],
                    )

                    # ---- mixed tile: scan 24 (banks 0-2, T=64) + offl 7 (bank 3)
                    h_ps = hpsp.tile([128, 2048], F32)
                    for j in range(NS):
                        o = (j // 8) * 512 + (j % 8) * W
                        mm(h_ps[:, o : o + W], NF + j, wo[half]["S"] + j * W, W)
                    for j in range(NO):
                        o = 3 * 512 + j * 2 * R_OFF
                        mm(h_ps[:, o : o + 2 * R_OFF],
                           NF + NS + j, wo[half]["O"] + j * 2 * R_OFF, 2 * R_OFF)
                    # scan: prefix-sum of w2*relu(h) over banks 0-2
                    prod = prodp.tile([128, (NS + 1) * W], F32)
                    nc.gpsimd.memset(prod[:, W - 1 : W], 0.0)
                    nc.vector._custom_dve(
                        PREFIX_SUM_OP,
                        out=prod[:, W:].rearrange("p (b c) -> p b c", b=3),
                        in0=w2e[:, half * NS * W : (half + 1) * NS * W].rearrange(
                            "p (b c) -> p b c", b=3),
                        in1=h_ps[:].rearrange("p (b q) -> p b q", b=4)[:, :3, : 8 * W],
                    )
                    ends = prod[:].rearrange("p (j c) -> p j c", c=W)[
                        :, :, W - 1 : W].rearrange("p j c -> p (j c)")
                    nc.gpsimd.tensor_sub(
                        out=pf[:, po + slot0 + NF : po + slot0 + NF + NS],
                        in0=ends[:, 1 : NS + 1], in1=ends[:, 0:NS],
                    )
                    # offl: |.|-drain bank 3, 4:1 tree on GpSimd, fp8 partials
                    habs = habsp.tile([128, NO * 2 * R_OFF + NO * R_OFF], BF16)
                    nc.scalar.activation(
                        out=habs[:, : NO * 2 * R_OFF],
                        in_=h_ps[:, 3 * 512 : 3 * 512 + NO * 2 * R_OFF],
                        func=mybir.ActivationFunctionType.Abs,
                    )
                    t0 = habs[:, : NO * 2 * R_OFF].rearrange(
                        "p (j k r) -> p j k r", j=NO, k=2)
                    t1 = habs[:, NO * 2 * R_OFF :].rearrange(
                        "p (j k r) -> p j k r", j=NO, k=2)
                    nc.gpsimd.tensor_add(out=t1, in0=t0[:, :, :, : R_OFF // 2],
                                         in1=t0[:, :, :, R_OFF // 2 :])
                    sc0 = hso + half * NO * OFF_SHIP
                    ship = hsb[:, sc0 : sc0 + NO * OFF_SHIP]
                    s4 = ship.rearrange("p (j k r) -> p j k r", j=NO, k=2)
                    nc.gpsimd.tensor_add(out=s4, in0=t1[:, :, :, : R_OFF // 4],
                                         in1=t1[:, :, :, R_OFF // 4 :])

                    # ---- small fold tile: 5 pathways, 1 bank
                    Ts = 2 * Rs
                    h_ps = hpsp.tile([128, NC * Ts], F32)
                    for j in range(NC):
                        mm(h_ps[:, j * Ts : (j + 1) * Ts],
                           NF + NS + NO + j, wo[half]["C"] + j * Ts, Ts)
                    habs = habsp.tile([128, NC * Ts], BF16)
                    nc.scalar.activation(out=habs[:], in_=h_ps[:],
                                         func=mybir.ActivationFunctionType.Abs)
                    prod = prodp.tile([128, (NC + 1) * Rs], F32)
                    nc.gpsimd.memset(prod[:, Rs - 1 : Rs], 0.0)
                    hr = habs[:].rearrange("p (j r) -> p j r", r=Ts)
                    nc.vector._custom_dve(
                        RAIL_DIFF_OP,
                        out=prod[:, Rs:].rearrange("p (j r) -> p j r", r=Rs),
                        in0=hr[:, :, :Rs], in1=hr[:, :, Rs:],
                    )
                    ends = prod[:].rearrange("p (j r) -> p j r", r=Rs)[
                        :, :, Rs - 1 : Rs].rearrange("p j r -> p (j r)")
                    nc.gpsimd.tensor_sub(
                        out=pf[:, po + slot0 + NF + NS + NO :][:, :NC],
                        in0=ends[:, 1 : NC + 1], in1=ends[:, 0:NC],
                    )
                # batched stores every 2 bt on the scalar ring
                if bt % 2 == 1:
                    nc.scalar.dma_start(
                        out=p_out[(bt - 1) * 128 : (bt + 1) * 128, :].rearrange(
                            "(k p) c -> p k c", p=128),
                        in_=pf[:].rearrange("p (k c) -> p k c", k=2),
                    )
                    nc.sync.dma_start(
                        out=h_out[(bt - 1) * 128 : (bt + 1) * 128, :].rearrange(
                            "(k p) c -> p k c", p=128),
                        in_=hsb[:].rearrange("p (k c) -> p k c", k=2),
                    )
    nc.finalize()
    return nc, wext_cols, wo


def _prep(W1, W2, halves, wext_cols, wo):
    W1T = np.transpose(np.asarray(W1, np.float32), (0, 2, 1))        # [P,V,W]
    wt = 0.8 * np.asarray(W2, np.float32)                            # [P,W]
    wext = np.zeros((V, wext_cols), np.float32)
    w2e_row = np.zeros(2 * NS * W, np.float32)

    def rails(p, R):
        pos = np.where(wt[p] > 0)[0]
        neg = np.where(wt[p] <= 0)[0]
        blk = np.zeros((V, 2 * R), np.float32)
        blk[:, : len(pos)] = W1T[p][:, pos] * np.abs(wt[p][pos])
        blk[:, R : R + len(neg)] = W1T[p][:, neg] * np.abs(wt[p][neg])
        return blk * S_RAIL

    for h in range(2):
        g = halves[h]
        c = wo[h]["F"]
        for p in g["f_main"]:
            wext[:, c : c + 2 * g["r_main"]] = rails(p, g["r_main"])
            c += 2 * g["r_main"]
        c = wo[h]["S"]
        for i, p in enumerate(g["scan"]):
            wext[:, c : c + W] = W1T[p] * S_SCAN
            w2e_row[h * NS * W + i * W : h * NS * W + (i + 1) * W] = wt[p]
            c += W
        c = wo[h]["O"]
        for p in g["offl"]:
            wext[:, c : c + 2 * R_OFF] = rails(p, R_OFF)
            c += 2 * R_OFF
        c = wo[h]["C"]
        for p in g["f_small"]:
            wext[:, c : c + 2 * g["r_small"]] = rails(p, g["r_small"])
            c += 2 * g["r_small"]
    wext_f8 = wext.astype(ml_dtypes.float8_e4m3)
    w2ext = np.ascontiguousarray(np.broadcast_to(
        w2e_row.astype(ml_dtypes.bfloat16)[None, :], (128, 2 * NS * W)))
    return wext_f8, w2ext


def _prep_xt(x_f8, slot_paths):
    """Pre-transpose per core into [v, (bt, slot, b)] fp8, slot = device order."""
    out = []
    for c in range(N_CORES):
        xc = x_f8[c * BSH : (c + 1) * BSH, :]
        xt = (
            xc.reshape(NBT, 128, P, V)[:, :, slot_paths, :]  # [bt, b, slot, v]
            .transpose(3, 0, 2, 1)                           # [v, bt, slot, b]
            .reshape(V, BSH * P)
        )
        out.append(np.ascontiguousarray(xt))
    return out


def kernel(x, W1, W2, gamma, beta, Wd, bd):
    global LAST_RESULTS
    x = np.ascontiguousarray(np.asarray(x, dtype=np.float32))
    W1 = np.asarray(W1, dtype=np.float32)
    W2 = np.asarray(W2, dtype=np.float32)

    halves = _plan(W2)
    sig = _geom_sig(halves)
    if _CACHE.get("sig") != sig:
        _CACHE["nc"], _CACHE["wc"], _CACHE["wo"] = _build_program(halves)
        _CACHE["sig"] = sig
    nc, wext_cols, wo = _CACHE["nc"], _CACHE["wc"], _CACHE["wo"]

    # device slot order -> original pathway ids
    slot_paths = np.concatenate([
        np.concatenate([g["f_main"], g["scan"], g["offl"], g["f_small"]])
        for g in halves
    ]).astype(np.int64)

    wext_f8, w2ext = _prep(W1, W2, halves, wext_cols, wo)
    x_f8 = x.astype(ml_dtypes.float8_e4m3)
    xts = _prep_xt(x_f8, slot_paths)
    in_maps = [
        {"xt": xts[c], "wext": wext_f8, "w2ext": w2ext}
        for c in range(N_CORES)
    ]
    res = run_bass_kernel_spmd(nc, in_maps, list(range(N_CORES)))
    LAST_RESULTS = res

    # ---- host finish ----
    # linear term q[b,p] = w2_p . z1 = x_bp . (W1T_p @ w2_p), batched BLAS
    W1T = np.transpose(W1, (0, 2, 1))
    U2 = np.einsum("pvw,pw->pv", W1T, W2).astype(np.float32)          # [P,V]
    xr = x.reshape(B, P, V)
    qall = np.einsum("bpv,pv->bp", xr, U2, optimize=True).astype(np.float64)

    ps = np.concatenate([res.results[c]["ps"] for c in range(N_CORES)],
                        axis=0).astype(np.float64)                    # [B,P] slots
    hs = np.concatenate([res.results[c]["hs"] for c in range(N_CORES)],
                        axis=0).astype(np.float64)                    # [B, 2*NO*18]

    z2 = np.empty((B, P), np.float64)
    for h in range(2):
        g = halves[h]
        s0 = h * 64
        # fold: z2 = 0.6 q + 0.5 * t1 / S_RAIL
        for idx, plist in ((0, g["f_main"]), (NF + NS + NO, g["f_small"])):
            sl = ps[:, s0 + idx : s0 + idx + len(plist)]
            z2[:, plist] = 0.6 * qall[:, plist] + 0.5 * sl / S_RAIL
        # scan: z2 = 0.2 q + slot / S_SCAN
        sl = ps[:, s0 + NF : s0 + NF + NS]
        z2[:, g["scan"]] = 0.2 * qall[:, g["scan"]] + sl / S_SCAN
        # offl: shipped fp8 partials [NO, 2, 18/2] pos/neg
        blk = hs[:, h * NO * OFF_SHIP : (h + 1) * NO * OFF_SHIP].reshape(
            B, NO, 2, OFF_SHIP // 2)
        t1 = blk[:, :, 0, :].sum(-1) - blk[:, :, 1, :].sum(-1)
        z2[:, g["offl"]] = 0.6 * qall[:, g["offl"]] + 0.5 * t1 / S_RAIL

    pvals = np.where(z2 >= 0, z2, 0.2 * z2)
    mean = pvals.mean(axis=0)
    var = pvals.var(axis=0)
    pn = (pvals - mean) / np.sqrt(var + BN_EPS) * np.asarray(gamma, np.float64) \
        + np.asarray(beta, np.float64)
    pn = pn / np.linalg.norm(pn)
    out = 1.0 / (1.0 + np.exp(-(pn @ np.asarray(Wd, np.float64)
                                + np.asarray(bd, np.float64))))
    return out.astype(np.float32)


# revision 17
# speedup vs baseline: 1.2666x; 1.2403x over previous
"""DeepHisCoM Trainium2 kernel (nn_DeepHisCoM_7017976562218).

Math (reference):
    xr = x.reshape(B, P, V)
    z1 = einsum('bpv,pwv->bpw', xr, W1);  h = leaky(z1)          # per-pathway Linear V->W
    z2 = einsum('bpw,pw->bp', h, W2);     pval = leaky(z2)       # per-pathway Linear W->1
    BN(batch stats) -> global L2 normalize -> sigmoid(pn @ Wd + bd)

Decomposition: with wt = 0.8*W2 and q = w2^T z1 (linear in x),
    z2 = 0.2*q + sum_t wt_t * relu(z1_t)
       = 0.6*q + 0.5 * sum_t sign(wt_t) * |wt_t * z1_t|          # relu(x) = (x+|x|)/2
The |.| form needs no relu engine pass and no sign bookkeeping on device:
columns are pre-scaled by |wt| on the host and laid out as a positive rail
and a negative rail; the device reduces  t1 = sum(pos rail) - sum(neg rail)
of |h~| and the host adds the 0.6*q linear term (one tiny batched GEMV,
0.17% of the model FLOPs, computed alongside BN/L2/sigmoid).

Device strategy (8 NeuronCores, batch-sharded 2048 rows/core), three
pathway routes chosen per-pathway by sign balance (host-planned from W2):
    - scan route (the ~48 most sign-unbalanced pathways): 64-col matmul
      (W1^T * S), VectorE fused prefix-sum of w2*relu(h) (custom DVE op),
      per-pathway sums via segment-boundary differences on GpSimd.
    - fold route (the ~66 most balanced): matmul of |wt|-prescaled rails
      [pos R | neg R]; ScalarE |.|-drains PSUM to bf16; VectorE runs a
      rail-difference prefix scan (cumsum(pos - neg), custom DVE op) over
      R cols/pathway instead of 64 -> ~2x less VectorE work; boundary
      differences on GpSimd give t1 directly.
    - offload route (~14 mid pathways): rails as above, but GpSimd
      tree-folds the |h~| rails 4:1 and ships fp8 partials to the host.
All routes write raw (pre-leaky) z2 parts; host applies leaky + BN.
"""

import os
import sys

import numpy as np

for _p in ("/opt/trn_rl_repo",):
    if _p not in sys.path and os.path.isdir(_p):
        sys.path.insert(0, _p)

import ml_dtypes

import concourse.bacc as bacc
import concourse.bass as bass
import concourse.mybir as mybir
from concourse import dve_ops
from concourse.bass_utils import run_bass_kernel_spmd
from concourse.dve_spec import AluOp, Spec, Src0, Src1, Zero, relu, scan
from concourse.tile import TileContext


def _register_op(name, spec_body, ref, sha):
    for op in dve_ops.OPS:
        if op.name == name:
            return op
    op = dve_ops.DveOp(name, Spec(body=spec_body, reference=ref),
                       subdim=False, uops_sha=sha)
    dve_ops.OPS.append(op)
    dve_ops._SUB_OPCODE_FOR_NAME[name] = dve_ops._CUSTOM_DVE_ROW_BASE + len(dve_ops.OPS) - 1
    dve_ops.CUSTOM_DVE_SPECS[name] = op.spec
    return op


# out[t] = running sum of in0[t] - in1[t]   (pos rail minus neg rail)
RAIL_DIFF_OP = _register_op(
    "RAIL_DIFF_SCAN_ANT",
    scan(AluOp.ADD, Src0 - Src1, init=Zero),
    lambda in0, in1, s0, s1, imm2: np.cumsum(
        in0.astype(np.float32) - in1.astype(np.float32), axis=-1),
    {"v3": "1a1aa71c909c123d", "v4": "cc23e684b94fd50d"},
)
# out[t] = running sum of in0[t] * relu(in1[t])
PREFIX_SUM_OP = _register_op(
    "STT_PREFIX_SUM_ANT",
    scan(AluOp.ADD, Src0 * relu(Src1), init=Zero),
    lambda in0, in1, s0, s1, imm2: np.cumsum(
        in0.astype(np.float32) * np.maximum(in1, 0), axis=-1),
    {"v3": "0179e875ac56dbc9", "v4": "d52b99774727e4db"},
)

P, V, W = 128, 128, 64
B = 16384
N_CORES = 8
BSH = B // N_CORES          # 2048 batch rows per core
NBT = BSH // 128            # 16 batch tiles per core
BN_EPS = 1e-5
F32 = mybir.dt.float32
BF16 = mybir.dt.bfloat16
F8 = mybir.dt.float8e4
S_SCAN = 8.0                # fp8 scale for scan-route W1^T columns
S_RAIL = 128.0              # fp8 scale for |wt|-prescaled rail columns

# per-half route sizes (128 pathways total, 64 per half)
NF = 28                     # fold pathways: 2 PSUM gens of 14 (7/bank)
NS = 24                     # scan pathways: gens of 16 (2 banks) + 8 (1 bank)
NO = 12                     # offload pathways: 1 gen, 6/bank, rails
R_OFF = 36                  # offload rail width (even, for the 2:1 tree)
OFF_SHIP = 2 * (R_OFF // 4) # fp8 partials shipped per offl pathway (18)

_CACHE = {}
LAST_RESULTS = None


def _plan(W2):
    """Route assignment + rail widths from the sign balance of W2."""
    wt = 0.8 * np.asarray(W2, np.float32)
    npos = (wt > 0).sum(1)
    bal = np.maximum(npos, W - npos)
    order = np.argsort(bal, kind="stable")
    fold = order[: 2 * NF]
    offl = order[2 * NF : 2 * (NF + NO)]
    scn = order[2 * (NF + NO) :]
    halves = []
    for h in range(2):
        f_h = fold[h * NF : (h + 1) * NF]
        s_h = scn[h * NS : (h + 1) * NS]
        o_h = offl[h * NO : (h + 1) * NO]
        r_f = int(bal[f_h].max())
        assert 7 * 2 * r_f <= 512 and R_OFF >= int(bal[o_h].max())
        halves.append({"fold": f_h, "r_f": r_f, "scan": s_h, "offl": o_h})
    return halves


def _geom_sig(halves):
    return tuple(h["r_f"] for h in halves)


def _build_program(halves):
    nc = bacc.Bacc()
    wext_cols = sum(
        2 * h["r_f"] * NF + NS * W + NO * 2 * R_OFF for h in halves
    )
    xt_in = nc.declare_dram_parameter("xt", [V, BSH * P], F8, isOutput=False)
    wext_in = nc.declare_dram_parameter("wext", [V, wext_cols], F8, isOutput=False)
    w2e_in = nc.declare_dram_parameter("w2ext", [128, 2 * NS * W], BF16, isOutput=False)
    p_out = nc.declare_dram_parameter("ps", [BSH, P], BF16, isOutput=True)
    h_out = nc.declare_dram_parameter("hs", [BSH, 2 * NO * OFF_SHIP], F8, isOutput=True)

    # per-half wext column offsets: [fold rails | scan | offl rails]
    wo = []
    off = 0
    for h in range(2):
        d = {"F": off}
        off += 2 * halves[h]["r_f"] * NF
        d["S"] = off
        off += NS * W
        d["O"] = off
        off += NO * 2 * R_OFF
        wo.append(d)
    assert off == wext_cols

    with TileContext(nc) as tc:
        with (
            tc.tile_pool(name="singles", bufs=1) as singles,
            tc.tile_pool(name="xt", bufs=4) as xtp,
            tc.tile_pool(name="prod", bufs=6) as prodp,
            tc.tile_pool(name="habs", bufs=4) as habsp,
            tc.tile_pool(name="habo", bufs=4) as habop,
            tc.tile_pool(name="hsb", bufs=2) as hsbp,
            tc.tile_pool(name="pf", bufs=2) as pfp,
            tc.tile_pool(name="hps", bufs=4, space="PSUM") as hpsp,
        ):
            wext = singles.tile([V, wext_cols], F8)
            w2e = singles.tile([128, 2 * NS * W], BF16)
            for h in range(2):
                c0 = wo[h]["F"]
                nc.scalar.dma_start(out=wext[:, c0 : wo[h]["S"]],
                                    in_=wext_in[:, c0 : wo[h]["S"]])
                nc.scalar.dma_start(out=w2e[:, h * NS * W : (h + 1) * NS * W],
                                    in_=w2e_in[:, h * NS * W : (h + 1) * NS * W])
            for h in range(2):
                c0 = wo[h]["S"]
                c1 = wo[h]["O"] + NO * 2 * R_OFF
                nc.scalar.dma_start(out=wext[:, c0:c1], in_=wext_in[:, c0:c1])

            for bt in range(NBT):
                xt = xtp.tile([128, 128 * 128], F8, tag="xt")
                base_col = bt * 128 * 128
                bounds = (0, 1792, 4096, 8192, 16384) if bt == 0 \
                    else (0, 8192, 16384)
                for c0, c1 in zip(bounds[:-1], bounds[1:]):
                    nc.sync.dma_start(
                        out=xt[:, c0:c1],
                        in_=xt_in[:, base_col + c0 : base_col + c1],
                    )
                if bt % 2 == 0:
                    pf = pfp.tile([128, 2 * P], BF16, tag="pf")
                    hsb = hsbp.tile([128, 2 * 2 * NO * OFF_SHIP], F8, tag="hsb")
                po = (bt % 2) * P
                hso = (bt % 2) * 2 * NO * OFF_SHIP
                for half in range(2):
                    g = halves[half]
                    xoff = half * 64 * 128
                    slot0 = half * 64
                    R = g["r_f"]
                    T = 2 * R

                    def mm(ps_ap, slot, cols0, ncol):
                        nc.tensor.matmul(
                            ps_ap,
                            lhsT=xt[:, xoff + slot * 128 : xoff + (slot + 1) * 128],
                            rhs=wext[:, cols0 : cols0 + ncol],
                            start=True, stop=True,
                        )

                    # ---- fold: 2 gens of 14 rails; Abs into one elastic habs
                    habs = habsp.tile([128, NF * T], BF16)
                    for gen in range(2):
                        h_ps = hpsp.tile([128, 1024], F32)
                        for j in range(14):
                            o = (j // 7) * 512 + (j % 7) * T
                            mm(h_ps[:, o : o + T], gen * 14 + j,
                               wo[half]["F"] + (gen * 14 + j) * T, T)
                        nc.scalar.activation(
                            out=habs[:, gen * 14 * T : (gen + 1) * 14 * T].rearrange(
                                "p (b q) -> p b q", b=2),
                            in_=h_ps[:].rearrange("p (b q) -> p b q", b=2)[:, :, : 7 * T],
                            func=mybir.ActivationFunctionType.Abs,
                        )
                    prod = prodp.tile([128, (NF + 1) * R], F32)
                    nc.gpsimd.memset(prod[:, R - 1 : R], 0.0)
                    hr = habs[:].rearrange("p (j r) -> p j r", r=T)
                    nc.vector._custom_dve(
                        RAIL_DIFF_OP,
                        out=prod[:, R:].rearrange("p (j r) -> p j r", r=R),
                        in0=hr[:, :, :R], in1=hr[:, :, R:],
                    )
                    ends = prod[:].rearrange("p (j r) -> p j r", r=R)[
                        :, :, R - 1 : R].rearrange("p j r -> p (j r)")
                    nc.gpsimd.tensor_sub(
                        out=pf[:, po + slot0 : po + slot0 + NF],
                        in0=ends[:, 1 : NF + 1], in1=ends[:, 0:NF],
                    )

                    # ---- scan: gens of 16 (2 banks) and 8 (1 bank)
                    for gen, ns_g, sbase in ((0, 16, 0), (1, 8, 16)):
                        h_ps = hpsp.tile([128, ns_g * 64], F32)
                        for j in range(ns_g):
                            o = (j // 8) * 512 + (j % 8) * W
                            mm(h_ps[:, o : o + W], NF + sbase + j,
                               wo[half]["S"] + (sbase + j) * W, W)
                        prod = prodp.tile([128, (ns_g + 1) * W], F32)
                        nc.gpsimd.memset(prod[:, W - 1 : W], 0.0)
                        c0 = half * NS * W + sbase * W
                        nc.vector._custom_dve(
                            PREFIX_SUM_OP,
                            out=prod[:, W:].rearrange("p (b c) -> p b c", b=ns_g // 8),
                            in0=w2e[:, c0 : c0 + ns_g * W].rearrange(
                                "p (b c) -> p b c", b=ns_g // 8),
                            in1=h_ps[:].rearrange("p (b q) -> p b q", b=ns_g // 8),
                        )
                        ends = prod[:].rearrange("p (j c) -> p j c", c=W)[
                            :, :, W - 1 : W].rearrange("p j c -> p (j c)")
                        nc.gpsimd.tensor_sub(
                            out=pf[:, po + slot0 + NF + sbase :][:, :ns_g],
                            in0=ends[:, 1 : ns_g + 1], in1=ends[:, 0:ns_g],
                        )

                    # ---- offl: 1 gen of 12 rails (6/bank), Pool 4:1 tree, fp8
                    h_ps = hpsp.tile([128, 1024], F32)
                    TO = 2 * R_OFF
                    for j in range(NO):
                        o = (j // 6) * 512 + (j % 6) * TO
                        mm(h_ps[:, o : o + TO], NF + NS + j,
                           wo[half]["O"] + j * TO, TO)
                    habo = habop.tile([128, NO * TO + NO * R_OFF], BF16)
                    nc.scalar.activation(
                        out=habo[:, : NO * TO].rearrange("p (b q) -> p b q", b=2),
                        in_=h_ps[:].rearrange("p (b q) -> p b q", b=2)[:, :, : 6 * TO],
                        func=mybir.ActivationFunctionType.Abs,
                    )
                    t0 = habo[:, : NO * TO].rearrange(
                        "p (j k r) -> p j k r", j=NO, k=2)
                    t1 = habo[:, NO * TO :].rearrange(
                        "p (j k r) -> p j k r", j=NO, k=2)
                    nc.gpsimd.tensor_add(out=t1, in0=t0[:, :, :, : R_OFF // 2],
                                         in1=t0[:, :, :, R_OFF // 2 :])
                    sc0 = hso + half * NO * OFF_SHIP
                    s4 = hsb[:, sc0 : sc0 + NO * OFF_SHIP].rearrange(
                        "p (j k r) -> p j k r", j=NO, k=2)
                    nc.gpsimd.tensor_add(out=s4, in0=t1[:, :, :, : R_OFF // 4],
                                         in1=t1[:, :, :, R_OFF // 4 :])
                if bt % 2 == 1:
                    nc.scalar.dma_start(
                        out=p_out[(bt - 1) * 128 : (bt + 1) * 128, :].rearrange(
                            "(k p) c -> p k c", p=128),
                        in_=pf[:].rearrange("p (k c) -> p k c", k=2),
                    )
                    nc.sync.dma_start(
                        out=h_out[(bt - 1) * 128 : (bt + 1) * 128, :].rearrange(
                            "(k p) c -> p k c", p=128),
                        in_=hsb[:].rearrange("p (k c) -> p k c", k=2),
                    )
    nc.finalize()
    return nc, wext_cols, wo


def _prep(W1, W2, halves, wext_cols, wo):
    W1T = np.transpose(np.asarray(W1, np.float32), (0, 2, 1))        # [P,V,W]
    wt = 0.8 * np.asarray(W2, np.float32)                            # [P,W]
    wext = np.zeros((V, wext_cols), np.float32)
    w2e_row = np.zeros(2 * NS * W, np.float32)

    def rails(p, R):
        pos = np.where(wt[p] > 0)[0]
        neg = np.where(wt[p] <= 0)[0]
        blk = np.zeros((V, 2 * R), np.float32)
        blk[:, : len(pos)] = W1T[p][:, pos] * np.abs(wt[p][pos])
        blk[:, R : R + len(neg)] = W1T[p][:, neg] * np.abs(wt[p][neg])
        return blk * S_RAIL

    for h in range(2):
        g = halves[h]
        c = wo[h]["F"]
        for p in g["fold"]:
            wext[:, c : c + 2 * g["r_f"]] = rails(p, g["r_f"])
            c += 2 * g["r_f"]
        c = wo[h]["S"]
        for i, p in enumerate(g["scan"]):
            wext[:, c : c + W] = W1T[p] * S_SCAN
            w2e_row[h * NS * W + i * W : h * NS * W + (i + 1) * W] = wt[p]
            c += W
        c = wo[h]["O"]
        for p in g["offl"]:
            wext[:, c : c + 2 * R_OFF] = rails(p, R_OFF)
            c += 2 * R_OFF
    wext_f8 = wext.astype(ml_dtypes.float8_e4m3)
    w2ext = np.ascontiguousarray(np.broadcast_to(
        w2e_row.astype(ml_dtypes.bfloat16)[None, :], (128, 2 * NS * W)))
    return wext_f8, w2ext


def _prep_xt(x_f8, slot_paths):
    """Pre-transpose per core into [v, (bt, slot, b)] fp8, slot = device order."""
    out = []
    for c in range(N_CORES):
        xc = x_f8[c * BSH : (c + 1) * BSH, :]
        xt = (
            xc.reshape(NBT, 128, P, V)[:, :, slot_paths, :]  # [bt, b, slot, v]
            .transpose(3, 0, 2, 1)                           # [v, bt, slot, b]
            .reshape(V, BSH * P)
        )
        out.append(np.ascontiguousarray(xt))
    return out


def kernel(x, W1, W2, gamma, beta, Wd, bd):
    global LAST_RESULTS
    x = np.ascontiguousarray(np.asarray(x, dtype=np.float32))
    W1 = np.asarray(W1, dtype=np.float32)
    W2 = np.asarray(W2, dtype=np.float32)

    halves = _plan(W2)
    sig = _geom_sig(halves)
    if _CACHE.get("sig") != sig:
        _CACHE["nc"], _CACHE["wc"], _CACHE["wo"] = _build_program(halves)
        _CACHE["sig"] = sig
    nc, wext_cols, wo = _CACHE["nc"], _CACHE["wc"], _CACHE["wo"]

    # device slot order -> original pathway ids
    slot_paths = np.concatenate([
        np.concatenate([g["fold"], g["scan"], g["offl"]]) for g in halves
    ]).astype(np.int64)

    wext_f8, w2ext = _prep(W1, W2, halves, wext_cols, wo)
    x_f8 = x.astype(ml_dtypes.float8_e4m3)
    xts = _prep_xt(x_f8, slot_paths)
    in_maps = [
        {"xt": xts[c], "wext": wext_f8, "w2ext": w2ext}
        for c in range(N_CORES)
    ]
    res = run_bass_kernel_spmd(nc, in_maps, list(range(N_CORES)))
    LAST_RESULTS = res

    # ---- host finish ----
    # linear term q[b,p] = w2_p . z1 = x_bp . (W1T_p @ w2_p), batched BLAS
    W1T = np.transpose(W1, (0, 2, 1))
    U2 = np.einsum("pvw,pw->pv", W1T, W2).astype(np.float32)          # [P,V]
    xr = x.reshape(B, P, V)
    qall = np.einsum("bpv,pv->bp", xr, U2, optimize=True).astype(np.float64)

    ps = np.concatenate([res.results[c]["ps"] for c in range(N_CORES)],
                        axis=0).astype(np.float64)                    # [B,P] slots
    hs = np.concatenate([res.results[c]["hs"] for c in range(N_CORES)],
                        axis=0).astype(np.float64)                    # [B, 2*NO*18]

    z2 = np.empty((B, P), np.float64)
    for h in range(2):
        g = halves[h]
        s0 = h * 64
        # fold: z2 = 0.6 q + 0.5 * t1 / S_RAIL
        sl = ps[:, s0 : s0 + NF]
        z2[:, g["fold"]] = 0.6 * qall[:, g["fold"]] + 0.5 * sl / S_RAIL
        # scan: z2 = 0.2 q + slot / S_SCAN
        sl = ps[:, s0 + NF : s0 + NF + NS]
        z2[:, g["scan"]] = 0.2 * qall[:, g["scan"]] + sl / S_SCAN
        # offl: shipped fp8 partials [NO, 2, 18/2] pos/neg
        blk = hs[:, h * NO * OFF_SHIP : (h + 1) * NO * OFF_SHIP].reshape(
            B, NO, 2, OFF_SHIP // 2)
        t1 = blk[:, :, 0, :].sum(-1) - blk[:, :, 1, :].sum(-1)
        z2[:, g["offl"]] = 0.6 * qall[:, g["offl"]] + 0.5 * t1 / S_RAIL

    pvals = np.where(z2 >= 0, z2, 0.2 * z2)
    mean = pvals.mean(axis=0)
    var = pvals.var(axis=0)
    pn = (pvals - mean) / np.sqrt(var + BN_EPS) * np.asarray(gamma, np.float64) \
        + np.asarray(beta, np.float64)
    pn = pn / np.linalg.norm(pn)
    out = 1.0 / (1.0 + np.exp(-(pn @ np.asarray(Wd, np.float64)
                                + np.asarray(bd, np.float64))))
    return out.astype(np.float32)
